# revision 1
# baseline (speedup 1.0000x reference)
"""BitLinear 1.58 Trainium2 Bass kernel — v2 (bf16/f16, N=512, engine rebalance).

Data-parallel over tokens: each of 8 cores takes 1024 tokens + full weight.

Per-core pipeline (vs v1 baseline):
  - matmul moving dim 512 (full PSUM bank): 2048 matmuls, engine-bound
    instead of PE-SEQ/Ldweights-bound.
  - weight ternarize (osub-granular, 32 units of 128 rows): |w| row-sums on
    the Act engine via Abs+accum_out (output discarded into a shared
    scratch; Act is serial so the WAW is free); signs on Act with
    sga = sign(-w+0.5s) via scale=-1 into its own tile, sgb = sign(w+0.5s)
    written IN-PLACE into the low half of the staged w chunk (bf16 write
    pointer 2i trails the f32 read pointer 4i, so no clobber); 2t = sgb-sga
    on DVE in-place; xbar transpose from the same low half. A whole
    o-tile's 4 units are emitted at the top of the previous o-tile's
    matmul loop.
  - x-quant: absmax/y/rint on DVE; dequant q*inv -> f16 on Pool, written
    in-place into the low half of the staged x chunk; transposed from
    there.
  - eviction scale row broadcast via ones-matmul into PSUM; evictions on
    DVE; out-stores on the Pool swdge; loads on SP hwdge; transposes on
    the Act hwdge.
"""
import sys

sys.path.insert(0, "/opt/trn_rl_repo")

import numpy as np

B, S, D_IN, D_OUT = 4, 2048, 4096, 4096
N_CORES = 8
M_TOT = B * S
M_C = M_TOT // N_CORES

P = 128
G = 64
OT = 512                        # columns per o-tile (full psum bank)
MAGIC = float(1.5 * 2.0 ** 23)  # fp32 round-to-nearest-even trick
EPS = 1e-5
QMAX = 127.0
INV_QMAX = float(np.float32(1.0 / 127.0))

_cache = {}


def _build(M, K, O, reps=1):
    import concourse.bass as bass
    import concourse.tile as tile
    from concourse import bacc, mybir

    f32 = mybir.dt.float32
    f16 = mybir.dt.float16
    bf16 = mybir.dt.bfloat16
    Alu = mybir.AluOpType
    Act = mybir.ActivationFunctionType
    Ax = mybir.AxisListType

    K2 = K // 2                 # 2048
    KSUB = K // P               # 32
    MB = M // P                 # 8
    NOT = O // OT               # 8
    OSUB = OT // P              # 4
    NU = O // P                 # 32 osub units
    NG2 = K2 // G               # 32 groups per half

    nc = bacc.Bacc("TRN2", target_bir_lowering=False, num_devices=1)
    x = nc.dram_tensor("x", [M, K], f32, kind="ExternalInput")
    w = nc.dram_tensor("w", [O, K], f32, kind="ExternalInput")
    out = nc.dram_tensor("out", [M, O], f32, kind="ExternalOutput")
    s_scr = nc.dram_tensor("s_scr", [O, 1], f32, kind="Internal")

    xap, wap, oap = x.ap(), w.ap(), out.ap()

    with tile.TileContext(nc) as tc:
        with (
            tc.tile_pool(name="xq", bufs=1) as xq_pool,
            tc.tile_pool(name="stage", bufs=4) as stage,
            tc.tile_pool(name="xstage", bufs=2) as xstage,
            tc.tile_pool(name="xq16", bufs=3) as xq16_pool,
            tc.tile_pool(name="sg", bufs=6) as sg_pool,
            tc.tile_pool(name="tt", bufs=2) as tt_pool,
            tc.tile_pool(name="small", bufs=3) as small,
            tc.tile_pool(name="ev", bufs=1) as ev_pool,
            tc.tile_pool(name="sb", bufs=2) as sb_pool,
            tc.tile_pool(name="ps", bufs=7, space="PSUM") as ps_pool,
        ):
            tt_tiles = {}
            w_stage = {}

            def wload_u(u):
                """Stage the 128 weight rows of osub unit u (2 chunks)."""
                if u >= NU:
                    return
                o0 = u * P
                tiles = []
                for h in range(2):
                    wh = stage.tile([P, K2], f32, tag="stage",
                                    name=f"wh{u}_{h}")
                    nc.sync.dma_start(
                        wh[:], wap[o0:o0 + P, h * K2:(h + 1) * K2])
                    tiles.append(wh)
                w_stage[u] = tiles

            w_bp = {}

            def wcompute_a(u):
                """Phase A: |w| row-sums -> bp, one pipeline slot ahead of
                the sign passes so Act never waits on the bp round-trip."""
                o0 = u * P
                whs = w_stage[u]
                # two-stage |w| row-sum (64-wide groups, then the group
                # sums) keeps s near the f32-tree-sum the reference uses
                sh = small.tile([P, 2], f32, tag="sh")
                for h in range(2):
                    gs = small.tile([P, NG2], f32, tag="gs")
                    nc.vector.tensor_reduce(
                        gs[:], whs[h].rearrange("p (g e) -> p g e", e=G),
                        Ax.X, Alu.add, apply_absolute_value=True)
                    nc.vector.tensor_reduce(sh[:, h:h + 1], gs[:], Ax.X,
                                            Alu.add)
                # bp = 0.5 * max(mean|row|, eps); sign bias + eviction
                # scale; smalls on gpsimd to keep the DVE queue clean
                ssum = small.tile([P, 1], f32, tag="ssum")
                nc.gpsimd.tensor_tensor(ssum[:], sh[:, 0:1], sh[:, 1:2],
                                        Alu.add)
                bp = small.tile([P, 1], f32, tag="bp")
                nc.gpsimd.tensor_scalar(bp[:], ssum[:],
                                        float(np.float32(0.5 / K)),
                                        0.5 * EPS, Alu.mult, Alu.max)
                nc.sync.dma_start(s_scr.ap()[o0:o0 + P, :], bp[:])
                w_bp[u] = bp

            def wcompute_b(u):
                """Phase B: 2t = sign(w-0.5s) + sign(w+0.5s) = sgb - sga
                with sga = sign(-w+0.5s), sgb = sign(w+0.5s); exact bf16.
                sgb has its own tile so the staged chunk is released at the
                sign reads (not held through subtract + transpose)."""
                ot, osub = divmod(u, OSUB)
                if osub == 0:
                    tt_tiles[ot] = tt_pool.tile([P, KSUB, OT], bf16,
                                                tag="tt", name=f"tt{ot}")
                ttl = tt_tiles[ot]
                whs = w_stage.pop(u)
                bp = w_bp.pop(u)
                for h in range(2):
                    sga = sg_pool.tile([P, K2], bf16, tag="sg",
                                       name=f"sga{u}_{h}")
                    nc.scalar.activation(out=sga[:], in_=whs[h][:],
                                         func=Act.Sign, bias=bp[:],
                                         scale=-1.0)
                    sgb = sg_pool.tile([P, K2], bf16, tag="sg",
                                       name=f"sgb{u}_{h}")
                    nc.scalar.activation(out=sgb[:], in_=whs[h][:],
                                         func=Act.Sign, bias=bp[:],
                                         scale=1.0)
                    nc.vector.tensor_tensor(sgb[:], sgb[:], sga[:],
                                            Alu.subtract)
                    nc.scalar.dma_start_transpose(
                        ttl[:, h * (KSUB // 2):(h + 1) * (KSUB // 2),
                            osub * P:(osub + 1) * P], sgb[:])

            # -------- activation quantization + transpose (one (mb,h)) ----
            xq_tiles = [xq_pool.tile([P, KSUB, P], f16, tag=f"xq{mb}",
                                     name=f"xq{mb}") for mb in range(MB)]

            K4 = K // 4
            NG4 = K4 // G

            def xquant(mb, q, pooly=False):
                xt = xstage.tile([P, K4], f32, tag="xstage",
                                 name=f"xt{mb}_{q}")
                nc.sync.dma_start(
                    xt[:], xap[mb * P:(mb + 1) * P, q * K4:(q + 1) * K4])
                xg = xt.rearrange("p (g e) -> p g e", e=G)
                am = small.tile([P, NG4], f32, tag="am")
                nc.vector.tensor_reduce(am[:], xg, Ax.X, Alu.max,
                                        apply_absolute_value=True)
                am2 = small.tile([P, NG4], f32, tag="am2")
                nc.vector.tensor_scalar(am2[:], am[:], EPS, None, Alu.max)
                rc = small.tile([P, NG4], f32, tag="rc")
                nc.vector.reciprocal(rc[:], am2[:])
                scale = small.tile([P, NG4], f32, tag="scale")
                nc.vector.tensor_scalar(scale[:], rc[:], QMAX, None,
                                        Alu.mult)
                inv = small.tile([P, NG4], f32, tag="inv")
                nc.vector.tensor_scalar(inv[:], am2[:], INV_QMAX, None,
                                        Alu.mult)
                yeng = nc.gpsimd if pooly else nc.vector
                yeng.tensor_tensor(
                    xg, xg, scale[:, :, None].to_broadcast((P, NG4, G)),
                    Alu.mult)
                nc.vector.tensor_scalar(xt[:], xt[:], MAGIC, MAGIC,
                                        Alu.add, Alu.subtract)
                xq16 = xq16_pool.tile([P, K4], f16, tag="xq16",
                                      name=f"xq16_{mb}_{q}")
                nc.gpsimd.tensor_tensor(
                    xq16.rearrange("p (g e) -> p g e", e=G), xg,
                    inv[:, :, None].to_broadcast((P, NG4, G)), Alu.mult)
                nc.scalar.dma_start_transpose(
                    xq_tiles[mb][:, q * (KSUB // 4):(q + 1) * (KSUB // 4),
                                 :], xq16)

            def load_sbc(ot):
                """Baseline-style broadcast: 0-stride-partition DMA of the
                o-tile's 512 eviction scales from DRAM scratch."""
                s_base = s_scr.ap()[ot * OT:(ot + 1) * OT, 0]
                s_bc_ap = bass.AP(tensor=s_base.tensor,
                                  offset=s_base.offset,
                                  ap=[[0, P], *s_base.ap])
                sbc = sb_pool.tile([P, OT], f32)
                nc.gpsimd.dma_start(sbc[:], s_bc_ap)
                return sbc

            # -------- emission schedule --------
            import contextlib
            rep_ctx = tc.For_i(0, reps) if reps > 1 else \
                contextlib.nullcontext()
            # steady cadence per unit u: B(u); load(u+2); A(u+1) — the
            # stage ring (4 chunks = 2 units) and the one-slot-early bp
            # make each phase's inputs ready when its engine reaches it.
            with rep_ctx:
                def cadence(u):
                    wcompute_b(u)
                    wload_u(u + 2)
                    if u + 1 < NU:
                        wcompute_a(u + 1)

                xq_sched = [(mb, q) for mb in range(MB) for q in range(4)]

                wload_u(0)
                wcompute_a(0)
                wload_u(1)
                for u in range(OSUB):        # o-tile 0 ternarize
                    cadence(u)
                    for _ in range(4):
                        xquant(*xq_sched.pop(0))

                sbc = load_sbc(0)
                sbc_next = None
                for ot in range(NOT):
                    ttl = tt_tiles.pop(ot)
                    pend = (list(range((ot + 1) * OSUB, (ot + 2) * OSUB))
                            if ot + 1 < NOT else [])

                    for mb in range(MB):
                        if mb in (0, 1, 3, 5) and pend:
                            cadence(pend.pop(0))
                        for _ in range(min(3, len(xq_sched))):
                            xquant(*xq_sched.pop(0))
                        ps = ps_pool.tile([P, OT], f32)
                        for ks in range(KSUB):
                            nc.tensor.matmul(
                                ps[:], xq_tiles[mb][:, ks, :], ttl[:, ks, :],
                                start=(ks == 0), stop=(ks == KSUB - 1))
                        ev = ev_pool.tile([P, OT], f32)
                        nc.vector.tensor_tensor(ev[:], ps[:], sbc[:], Alu.mult)
                        nc.gpsimd.dma_start(
                            oap[mb * P:(mb + 1) * P, ot * OT:(ot + 1) * OT],
                            ev[:])
                        if mb == 6 and ot + 1 < NOT:
                            sbc_next = load_sbc(ot + 1)
                    sbc = sbc_next


    nc.compile()
    return nc


def _get_nc():
    if "nc" not in _cache:
        _cache["nc"] = _build(M_C, D_IN, D_OUT)
    return _cache["nc"]


def run(x, weight, trace=False):
    """Run on 8 NeuronCores; returns (full output [B,S,D_OUT], results obj)."""
    from concourse.bass_utils import run_bass_kernel_spmd

    x = np.ascontiguousarray(np.asarray(x, dtype=np.float32))
    w = np.ascontiguousarray(np.asarray(weight, dtype=np.float32))
    assert x.shape == (B, S, D_IN) and w.shape == (D_OUT, D_IN)
    xf = x.reshape(M_TOT, D_IN)
    nc = _get_nc()
    in_maps = [
        {"x": np.ascontiguousarray(xf[c * M_C:(c + 1) * M_C]), "w": w}
        for c in range(N_CORES)
    ]
    res = run_bass_kernel_spmd(nc, in_maps, core_ids=list(range(N_CORES)),
                               trace=trace)
    outf = np.concatenate([res.results[c]["out"] for c in range(N_CORES)],
                          axis=0)
    return outf.reshape(B, S, D_OUT), res


def kernel(x, weight):
    out, _ = run(x, weight)
    return out



# revision 5
# speedup vs baseline: 1.2728x; 1.2728x over previous
"""BitLinear 1.58 Trainium2 Bass kernel — v3.

2D sharding per the tensor-parallel hint: 4 token-quarters x 2 weight-
halves. Each core: x_shard [2048, 4096], w_half [2048, 4096] ->
out [2048, 2048] f16 (host converts to f32 and reassembles).

Per-core pipeline:
  - Ternary weights live RESIDENT in SBUF as fp8e4 {-2,0,2} (or {-1,0,1}
    for DVE-ternarized units), built once: per 128-row unit, |w| row-mean
    on DVE, then signs on Act (even units: 2x Sign) or DVE (odd units:
    is_ge/is_lt tensor_scalar with per-partition threshold), subtract,
    bf16 DMA-transpose k-major, fp8 convert on DVE. Row scale 0.5*s (or
    s) round-trips through DRAM and broadcast-loads as the eviction
    scale per o-tile.
  - x quant per 128-token block: group-64 absmax on DVE, scale mult and
    dequant-to-f16 on Pool, magic round on DVE, f16 DMA-transpose.
  - Matmuls: f16 stationary (xq^T) x fp8 moving (ternary w^T), N=512,
    mb-outer with a 3-mb staggered ramp so the PE starts ~40us in and
    never waits on weight prep. Evictions ps*scale -> f16 on DVE.
  - Queues: w loads + all transposes on SP hwdge; x loads, small stores
    and out stores on Pool swdge; Act engine runs signs only.
"""
import sys

sys.path.insert(0, "/opt/trn_rl_repo")

import numpy as np

B, S, D_IN, D_OUT = 4, 2048, 4096, 4096
N_CORES = 8
TQ = 4                          # token shards
WQ = 2                          # weight-row shards
M_TOT = B * S
M_C = M_TOT // TQ               # 2048 tokens per core
O_C = D_OUT // WQ               # 2048 out cols per core
K = D_IN                        # 4096

P = 128
G = 64
OT = 512                        # o-tile (one psum bank)
MAGIC = float(1.5 * 2.0 ** 23)
EPS = 1e-5
QMAX = 127.0
INV_QMAX = float(np.float32(1.0 / 127.0))

MB = M_C // P                   # 16 token blocks
NU = O_C // P                   # 16 weight units
NOT_ = O_C // OT                # 4 o-tiles
KSUB = K // P                   # 32
NG = K // G                     # 64 groups per w row
KH = K // 2                     # 2048: x processed in halves
NGH = KH // G                   # 32

_cache = {}


def _build():
    import concourse.bass as bass
    import concourse.tile as tile
    from concourse import bacc, mybir

    f32 = mybir.dt.float32
    f16 = mybir.dt.float16
    bf16 = mybir.dt.bfloat16
    fp8 = mybir.dt.float8e4
    Alu = mybir.AluOpType
    Act = mybir.ActivationFunctionType
    Ax = mybir.AxisListType

    nc = bacc.Bacc("TRN2", target_bir_lowering=False, num_devices=1)
    x = nc.dram_tensor("x", [M_C, K], f32, kind="ExternalInput")
    w = nc.dram_tensor("w", [O_C, K], f32, kind="ExternalInput")
    out = nc.dram_tensor("out", [M_C, O_C], f16, kind="ExternalOutput")
    s_scr = nc.dram_tensor("s_scr", [O_C, 1], f32, kind="Internal")

    xap, wap, oap = x.ap(), w.ap(), out.ap()

    with tile.TileContext(nc) as tc:
        with (
            tc.tile_pool(name="xq", bufs=5) as xq_pool,
            tc.tile_pool(name="tt", bufs=NOT_) as tt_pool,
            tc.tile_pool(name="tscr", bufs=1) as tscr_pool,
            tc.tile_pool(name="wstage", bufs=2) as wstage,
            tc.tile_pool(name="xstage", bufs=2) as xstage,
            tc.tile_pool(name="xq16", bufs=2) as xq16_pool,
            tc.tile_pool(name="sg", bufs=2) as sg_pool,
            tc.tile_pool(name="ev", bufs=2) as ev_pool,
            tc.tile_pool(name="sb", bufs=NOT_) as sb_pool,
            tc.tile_pool(name="small", bufs=4) as small,
            tc.tile_pool(name="ps", bufs=6, space="PSUM") as ps_pool,
        ):
            # resident ternary weight^T, one tile per o-tile
            tt_tiles = [tt_pool.tile([P, KSUB, OT], fp8, tag="tt",
                                     name=f"tt{ot}") for ot in range(NOT_)]
            xq_tiles = {}
            ev_tiles = {}
            w_stage = {}
            sbc_tiles = {}

            def w_load(u):
                wst = wstage.tile([P, K], f32, tag="wst", name=f"wst{u}")
                nc.sync.dma_start(wst[:], wap[u * P:(u + 1) * P, :])
                w_stage[u] = wst

            def w_chain(u):
                """Reduce |w|, ternarize, transpose, fp8-convert unit u.
                Even units ternarize on Act (2x Sign), odd on DVE."""
                use_act = (u % 2 == 0)
                wst = w_stage.pop(u)
                gs = small.tile([P, NG], f32, tag="gs")
                nc.vector.tensor_reduce(
                    gs[:], wst.rearrange("p (g e) -> p g e", e=G),
                    Ax.X, Alu.add, apply_absolute_value=True)
                s1 = small.tile([P, 1], f32, tag="s1")
                nc.vector.tensor_reduce(s1[:], gs[:], Ax.X, Alu.add)
                # bp = 0.5 * max(mean|row|, eps)
                bp = small.tile([P, 1], f32, tag="bp")
                nc.gpsimd.tensor_scalar(bp[:], s1[:],
                                        float(np.float32(0.5 / K)),
                                        0.5 * EPS, Alu.mult, Alu.max)
                if use_act:
                    # d = sign(w+bp) - sign(-w+bp) in {-2,0,2};
                    # eviction scale is bp = 0.5*s
                    nc.gpsimd.dma_start(s_scr.ap()[u * P:(u + 1) * P, :],
                                        bp[:])
                    sga = sg_pool.tile([P, K], bf16, tag="sg",
                                       name=f"sga{u}")
                    nc.scalar.activation(out=sga[:], in_=wst[:],
                                         func=Act.Sign, bias=bp[:],
                                         scale=-1.0)
                    sgb = sg_pool.tile([P, K], bf16, tag="sg",
                                       name=f"sgb{u}")
                    nc.scalar.activation(out=sgb[:], in_=wst[:],
                                         func=Act.Sign, bias=bp[:],
                                         scale=1.0)
                    nc.vector.tensor_tensor(sgb[:], sgb[:], sga[:],
                                            Alu.subtract)
                    d = sgb
                else:
                    # d = (w >= bp) - (w < -bp) in {-1,0,1};
                    # eviction scale is s = 2*bp
                    sf = small.tile([P, 1], f32, tag="sf")
                    nc.gpsimd.tensor_scalar(sf[:], bp[:], 2.0, None,
                                            Alu.mult)
                    nc.gpsimd.dma_start(s_scr.ap()[u * P:(u + 1) * P, :],
                                        sf[:])
                    bpn = small.tile([P, 1], f32, tag="bpn")
                    nc.gpsimd.tensor_scalar(bpn[:], bp[:], -1.0, None,
                                            Alu.mult)
                    da = sg_pool.tile([P, K], bf16, tag="sg",
                                      name=f"da{u}")
                    nc.vector.tensor_scalar(da[:], wst[:], bp[:], None,
                                            Alu.is_ge)
                    db = sg_pool.tile([P, K], bf16, tag="sg",
                                      name=f"db{u}")
                    nc.vector.tensor_scalar(db[:], wst[:], bpn[:], None,
                                            Alu.is_lt)
                    nc.vector.tensor_tensor(da[:], da[:], db[:],
                                            Alu.subtract)
                    d = da
                scr = tscr_pool.tile([P, KSUB, P], bf16, tag="tscr",
                                     name=f"scr{u}")
                nc.sync.dma_start_transpose(scr[:], d[:])
                ot, ul = divmod(u, NU // NOT_)
                nc.vector.tensor_copy(
                    tt_tiles[ot][:, :, ul * P:(ul + 1) * P], scr[:])

            def load_sbc(ot):
                s_base = s_scr.ap()[ot * OT:(ot + 1) * OT, 0]
                s_bc_ap = bass.AP(tensor=s_base.tensor,
                                  offset=s_base.offset,
                                  ap=[[0, P], *s_base.ap])
                sbc = sb_pool.tile([P, OT], f32, tag="sb",
                                   name=f"sbc{ot}")
                nc.gpsimd.dma_start(sbc[:], s_bc_ap)
                sbc_tiles[ot] = sbc

            def x_quant(mb):
                xqt = xq_pool.tile([P, KSUB, P], f16, tag="xq",
                                   name=f"xq{mb}")
                xq_tiles[mb] = xqt
                for h in range(2):
                    xt = xstage.tile([P, KH], f32, tag="xst",
                                     name=f"xst{mb}_{h}")
                    nc.gpsimd.dma_start(
                        xt[:], xap[mb * P:(mb + 1) * P,
                                   h * KH:(h + 1) * KH])
                    xg = xt.rearrange("p (g e) -> p g e", e=G)
                    am = small.tile([P, NGH], f32, tag="am")
                    nc.vector.tensor_reduce(am[:], xg, Ax.X, Alu.max,
                                            apply_absolute_value=True)
                    am2 = small.tile([P, NGH], f32, tag="am2")
                    nc.vector.tensor_scalar(am2[:], am[:], EPS, None,
                                            Alu.max)
                    rc = small.tile([P, NGH], f32, tag="rc")
                    nc.vector.reciprocal(rc[:], am2[:])
                    scale = small.tile([P, NGH], f32, tag="scale")
                    nc.vector.tensor_scalar(scale[:], rc[:], QMAX, None,
                                            Alu.mult)
                    inv = small.tile([P, NGH], f32, tag="inv")
                    nc.vector.tensor_scalar(inv[:], am2[:], INV_QMAX,
                                            None, Alu.mult)
                    nc.gpsimd.tensor_tensor(
                        xg, xg,
                        scale[:, :, None].to_broadcast((P, NGH, G)),
                        Alu.mult)
                    nc.vector.tensor_scalar(xt[:], xt[:], MAGIC, MAGIC,
                                            Alu.add, Alu.subtract)
                    xq16 = xq16_pool.tile([P, KH], f16, tag="xq16",
                                          name=f"xq16_{mb}_{h}")
                    nc.gpsimd.tensor_tensor(
                        xq16.rearrange("p (g e) -> p g e", e=G), xg,
                        inv[:, :, None].to_broadcast((P, NGH, G)),
                        Alu.mult)
                    nc.sync.dma_start_transpose(
                        xqt[:, h * (KSUB // 2):(h + 1) * (KSUB // 2), :],
                        xq16[:])

            def mm_group(mb, ot):
                if ot == 0:
                    ev_tiles[mb] = ev_pool.tile([P, O_C], f16, tag="ev",
                                                name=f"ev{mb}")
                ps = ps_pool.tile([P, OT], f32)
                xqt = xq_tiles[mb]
                ttl = tt_tiles[ot]
                for ks in range(KSUB):
                    nc.tensor.matmul(ps[:], xqt[:, ks, :], ttl[:, ks, :],
                                     start=(ks == 0), stop=(ks == KSUB - 1))
                nc.vector.tensor_tensor(
                    ev_tiles[mb][:, ot * OT:(ot + 1) * OT], ps[:],
                    sbc_tiles[ot][:], Alu.mult)

            def ev_store(mb):
                ev = ev_tiles.pop(mb)
                nc.gpsimd.dma_start(oap[mb * P:(mb + 1) * P, :], ev[:])

            # ---------------- emission schedule ----------------
            # prologue: units 0-7 chains, x 0-4, sbc0/1
            w_load(0)
            w_load(1)
            w_chain(0)
            x_quant(0)
            w_load(2)
            w_chain(1)
            w_load(3)
            w_chain(2)
            x_quant(1)
            w_load(4)
            w_chain(3)
            load_sbc(0)
            w_load(5)
            w_chain(4)
            x_quant(2)
            w_load(6)
            w_chain(5)
            w_load(7)
            w_chain(6)
            x_quant(3)
            w_load(8)
            w_chain(7)
            load_sbc(1)
            x_quant(4)

            # ramp: 3-mb stagger over o-tiles while units 8-15 build
            w_load(9)
            mm_group(0, 0)
            w_chain(8)
            mm_group(1, 0)
            w_load(10)
            w_chain(9)
            mm_group(2, 0)
            w_load(11)
            mm_group(0, 1)
            w_chain(10)
            mm_group(1, 1)
            w_load(12)
            w_chain(11)
            mm_group(2, 1)
            load_sbc(2)
            w_load(13)
            mm_group(0, 2)
            w_chain(12)
            mm_group(1, 2)
            w_load(14)
            w_chain(13)
            mm_group(2, 2)
            w_load(15)
            w_chain(14)
            w_chain(15)
            load_sbc(3)
            mm_group(0, 3)
            ev_store(0)
            mm_group(1, 3)
            ev_store(1)
            mm_group(2, 3)
            ev_store(2)
            x_quant(5)
            x_quant(6)

            # steady state
            for mb in range(3, MB):
                for ot in range(NOT_):
                    mm_group(mb, ot)
                    if ot == 1 and mb + 4 < MB:
                        x_quant(mb + 4)
                ev_store(mb)

    nc.compile()
    return nc


def _get_nc():
    if "nc" not in _cache:
        _cache["nc"] = _build()
    return _cache["nc"]


def run(x, weight, trace=False):
    """Run on 8 NeuronCores; returns (full output [B,S,D_OUT], results)."""
    from concourse.bass_utils import run_bass_kernel_spmd

    x = np.ascontiguousarray(np.asarray(x, dtype=np.float32))
    w = np.ascontiguousarray(np.asarray(weight, dtype=np.float32))
    assert x.shape == (B, S, D_IN) and w.shape == (D_OUT, D_IN)
    xf = x.reshape(M_TOT, D_IN)
    nc = _get_nc()
    in_maps = []
    for c in range(N_CORES):
        tq, wq = c % TQ, c // TQ
        in_maps.append({
            "x": np.ascontiguousarray(xf[tq * M_C:(tq + 1) * M_C]),
            "w": np.ascontiguousarray(w[wq * O_C:(wq + 1) * O_C]),
        })
    res = run_bass_kernel_spmd(nc, in_maps, core_ids=list(range(N_CORES)),
                               trace=trace)
    outf = np.empty((M_TOT, D_OUT), dtype=np.float32)
    for c in range(N_CORES):
        tq, wq = c % TQ, c // TQ
        outf[tq * M_C:(tq + 1) * M_C,
             wq * O_C:(wq + 1) * O_C] = res.results[c]["out"]
    return outf.reshape(B, S, D_OUT), res


def kernel(x, weight):
    out, _ = run(x, weight)
    return out


# revision 8
# speedup vs baseline: 1.6771x; 1.3176x over previous
"""BitLinear 1.58 Trainium2 Bass kernel — v4.

2D sharding: 4 token-quarters x 2 weight-halves. Each core:
x_shard [2048, 4096], w_half [2048, 4096] -> outT [2048 o, 2048 m] f16
(host transposes each core's block and reassembles the full f32 output).

Key structure (vs v3): the TERNARY WEIGHT is the matmul STATIONARY
operand (fp8e4 {-2,0,2}, resident 8MB, LDWEIGHTS hidden + FWL), and the
quantized activation x^T is the MOVING operand (f16 streams 1 col/cycle
= 213ns/MM at N=512; fp8 moving measured ~1.2 cyc/col — avoided).
Output tiles come out transposed [o, m], so the per-row scale 0.5*s is a
per-PARTITION scalar: evictions run on the Scalar engine as
Copy(ps * bp) -> f16, no DRAM scale round-trip.

Per-core pipeline:
  - 16 weight units of 128 rows, processed in 2KB halves: |w| row-mean
    (DVE two-stage), ternary d = sign(w+bp)+sign(w-bp) in {-2,0,2}:
    Act units via 2x Sign + DVE subtract; DVE units via 2-op
    tensor_scalar (w>=bp)*2 / (w<-bp)*2 + subtract. bf16 transpose on
    the SP hwdge ring, fp8 convert on Act.
  - x quant per 128-token block (halves): group-64 absmax/scales on DVE,
    q16 = x*scale -> f16 on Pool, magic-round + dequant in f16 on DVE
    (2x rate), f16 transpose into the 4-block "mquad" moving tile.
  - MM: 8 blocks (mquad 0-3) x (unit-half L/H); per (mq, u): 32
    accumulating MMs tt_u[:,ks,:] x xq[mq][:,ks,:] -> psum [o128, m512],
    Act eviction, store to outT.
"""
import sys

sys.path.insert(0, "/opt/trn_rl_repo")

import numpy as np

B, S, D_IN, D_OUT = 4, 2048, 4096, 4096
N_CORES = 8
TQ = 4
WQ = 2
M_TOT = B * S
M_C = M_TOT // TQ               # 2048 tokens per core
O_C = D_OUT // WQ               # 2048 out cols per core
K = D_IN

P = 128
G = 64
MW = 512                        # moving m-width (one psum bank)
MAGIC16 = float(1.5 * 2.0 ** 10)
EPS = 1e-5
QMAX = 127.0
INV_QMAX = float(np.float32(1.0 / 127.0))

MB = M_C // P                   # 16 token blocks
NU = O_C // P                   # 16 weight units
NMQ = M_C // MW                 # 4 mquads
KSUB = K // P                   # 32
KH = K // 2                     # 2048
NGH = KH // G                   # 32 x-quant groups per half
NGW = KH // G                   # 32 w-sum groups per half

DVE_UNITS = {5, 7, 9, 11, 13, 15}

_cache = {}


def _build():
    import concourse.tile as tile
    from concourse import bacc, mybir

    f32 = mybir.dt.float32
    f16 = mybir.dt.float16
    bf16 = mybir.dt.bfloat16
    fp8 = mybir.dt.float8e4
    Alu = mybir.AluOpType
    Act = mybir.ActivationFunctionType
    Ax = mybir.AxisListType

    nc = bacc.Bacc("TRN2", target_bir_lowering=False, num_devices=1)
    x = nc.dram_tensor("x", [M_C, K], f32, kind="ExternalInput")
    w = nc.dram_tensor("w", [O_C, K], f32, kind="ExternalInput")
    outT = nc.dram_tensor("outT", [O_C, M_C], f16, kind="ExternalOutput")

    xap, wap, oap = x.ap(), w.ap(), outT.ap()

    with tile.TileContext(nc) as tc:
        with (
            tc.tile_pool(name="tt", bufs=NU) as tt_pool,
            tc.tile_pool(name="xq", bufs=2) as xq_pool,
            tc.tile_pool(name="wst", bufs=3) as wstage,
            tc.tile_pool(name="xst", bufs=2) as xstage,
            tc.tile_pool(name="xq16", bufs=2) as xq16_pool,
            tc.tile_pool(name="sg", bufs=3) as sg_pool,
            tc.tile_pool(name="tscr", bufs=1) as tscr_pool,
            tc.tile_pool(name="evq", bufs=4) as ev_pool,
            tc.tile_pool(name="bp", bufs=NU) as bp_pool,
            tc.tile_pool(name="small", bufs=4) as small,
            tc.tile_pool(name="ps", bufs=6, space="PSUM") as ps_pool,
        ):
            tt_tiles = {}
            bp_tiles = {}
            xq_tiles = {}
            w_stage = {}

            def w_load(u):
                ts = []
                for h in range(2):
                    wst = wstage.tile([P, KH], f32, tag="wst",
                                      name=f"wst{u}_{h}")
                    nc.sync.dma_start(
                        wst[:], wap[u * P:(u + 1) * P,
                                    h * KH:(h + 1) * KH])
                    ts.append(wst)
                w_stage[u] = ts

            def w_chain(u):
                use_dve = u in DVE_UNITS
                whs = w_stage.pop(u)
                sh = small.tile([P, 2], f32, tag="sh")
                for h in range(2):
                    gs = small.tile([P, NGW], f32, tag="gs")
                    nc.vector.tensor_reduce(
                        gs[:], whs[h].rearrange("p (g e) -> p g e", e=G),
                        Ax.X, Alu.add, apply_absolute_value=True)
                    nc.vector.tensor_reduce(sh[:, h:h + 1], gs[:], Ax.X,
                                            Alu.add)
                s1 = small.tile([P, 1], f32, tag="s1")
                nc.gpsimd.tensor_tensor(s1[:], sh[:, 0:1], sh[:, 1:2],
                                        Alu.add)
                # bp = 0.5 * max(mean|row|, eps); also the eviction scale
                # (d is in {-2,0,2} for both ternarize paths)
                bp = bp_pool.tile([P, 1], f32, tag="bp", name=f"bp{u}")
                nc.gpsimd.tensor_scalar(bp[:], s1[:],
                                        float(np.float32(0.5 / K)),
                                        0.5 * EPS, Alu.mult, Alu.max)
                bp_tiles[u] = bp
                if use_dve:
                    bpn = small.tile([P, 1], f32, tag="bpn")
                    nc.gpsimd.tensor_scalar(bpn[:], bp[:], -1.0, None,
                                            Alu.mult)
                scr = tscr_pool.tile([P, KSUB, P], bf16, tag="tscr",
                                     name=f"scr{u}")
                for h in range(2):
                    if use_dve:
                        da = sg_pool.tile([P, KH], bf16, tag="sg",
                                          name=f"da{u}_{h}")
                        nc.vector.tensor_scalar(da[:], whs[h][:], bp[:],
                                                2.0, Alu.is_ge, Alu.mult)
                        db = sg_pool.tile([P, KH], bf16, tag="sg",
                                          name=f"db{u}_{h}")
                        nc.vector.tensor_scalar(db[:], whs[h][:], bpn[:],
                                                2.0, Alu.is_lt, Alu.mult)
                        nc.vector.tensor_tensor(da[:], da[:], db[:],
                                                Alu.subtract)
                        d = da
                    else:
                        sga = sg_pool.tile([P, KH], bf16, tag="sg",
                                           name=f"sga{u}_{h}")
                        nc.scalar.activation(out=sga[:], in_=whs[h][:],
                                             func=Act.Sign, bias=bp[:],
                                             scale=-1.0)
                        sgb = sg_pool.tile([P, KH], bf16, tag="sg",
                                           name=f"sgb{u}_{h}")
                        nc.scalar.activation(out=sgb[:], in_=whs[h][:],
                                             func=Act.Sign, bias=bp[:],
                                             scale=1.0)
                        nc.vector.tensor_tensor(sgb[:], sgb[:], sga[:],
                                                Alu.subtract)
                        d = sgb
                    nc.sync.dma_start_transpose(
                        scr[:, h * (KSUB // 2):(h + 1) * (KSUB // 2), :],
                        d[:])
                ttu = tt_pool.tile([P, KSUB, P], fp8, tag="tt",
                                   name=f"tt{u}")
                nc.scalar.activation(out=ttu[:], in_=scr[:],
                                     func=Act.Copy)
                tt_tiles[u] = ttu

            def x_quant(mb):
                mq, ml = divmod(mb, NMQ)
                if ml == 0:
                    xq_tiles[mq] = xq_pool.tile([P, KSUB, MW], f16,
                                                tag="xq", name=f"xq{mq}")
                xqt = xq_tiles[mq]
                for h in range(2):
                    xt = xstage.tile([P, KH], f32, tag="xst",
                                     name=f"xst{mb}_{h}")
                    nc.gpsimd.dma_start(
                        xt[:], xap[mb * P:(mb + 1) * P,
                                   h * KH:(h + 1) * KH])
                    xg = xt.rearrange("p (g e) -> p g e", e=G)
                    am = small.tile([P, NGH], f32, tag="am")
                    nc.vector.tensor_reduce(am[:], xg, Ax.X, Alu.max,
                                            apply_absolute_value=True)
                    am2 = small.tile([P, NGH], f32, tag="am2")
                    nc.vector.tensor_scalar(am2[:], am[:], EPS, None,
                                            Alu.max)
                    rc = small.tile([P, NGH], f32, tag="rc")
                    nc.vector.reciprocal(rc[:], am2[:])
                    scale = small.tile([P, NGH], f32, tag="scale")
                    nc.vector.tensor_scalar(scale[:], rc[:], QMAX, None,
                                            Alu.mult)
                    inv = small.tile([P, NGH], f16, tag="inv")
                    nc.vector.tensor_scalar(inv[:], am2[:], INV_QMAX,
                                            None, Alu.mult)
                    q16 = xq16_pool.tile([P, KH], f16, tag="xq16",
                                         name=f"q16_{mb}_{h}")
                    qg = q16.rearrange("p (g e) -> p g e", e=G)
                    nc.gpsimd.tensor_tensor(
                        qg, xg,
                        scale[:, :, None].to_broadcast((P, NGH, G)),
                        Alu.mult)
                    nc.vector.tensor_scalar(q16[:], q16[:], MAGIC16,
                                            MAGIC16, Alu.add,
                                            Alu.subtract)
                    nc.vector.tensor_tensor(
                        qg, qg,
                        inv[:, :, None].to_broadcast((P, NGH, G)),
                        Alu.mult)
                    nc.sync.dma_start_transpose(
                        xqt[:, h * (KSUB // 2):(h + 1) * (KSUB // 2),
                            ml * P:(ml + 1) * P], q16[:])

            def mm_group(mq, u):
                ps = ps_pool.tile([P, MW], f32)
                xqt = xq_tiles[mq]
                ttu = tt_tiles[u]
                for ks in range(KSUB):
                    nc.tensor.matmul(ps[:], ttu[:, ks, :], xqt[:, ks, :],
                                     start=(ks == 0),
                                     stop=(ks == KSUB - 1))
                ev = ev_pool.tile([P, MW], f16, tag="evq")
                nc.scalar.activation(out=ev[:], in_=ps[:], func=Act.Copy,
                                     scale=bp_tiles[u])
                nc.gpsimd.dma_start(
                    oap[u * P:(u + 1) * P, mq * MW:(mq + 1) * MW], ev[:])

            # ---------------- emission schedule ----------------
            w_load(0)
            w_load(1)
            w_load(2)
            x_quant(0)
            w_chain(0)
            x_quant(1)
            w_load(3)
            w_chain(1)
            x_quant(2)
            w_load(4)
            w_chain(2)
            x_quant(3)
            w_load(5)
            w_chain(3)

            # global pending work, drained between matmul groups; chain u
            # is force-drained before the first group that reads tt[u],
            # and x_quant(mb) before the first group reading its mquad
            pending = [
                ("l", 6), ("c", 4), ("l", 7), ("c", 5),
                ("l", 8), ("c", 6), ("l", 9), ("c", 7),
                ("l", 10), ("c", 8), ("l", 11), ("c", 9),
                ("x", 4), ("l", 12), ("c", 10), ("l", 13), ("c", 11),
                ("l", 14), ("c", 12), ("l", 15), ("c", 13),
                ("c", 14), ("x", 5), ("c", 15), ("x", 6), ("x", 7),
                ("x", 8), ("x", 9), ("x", 10), ("x", 11),
                ("x", 12), ("x", 13), ("x", 14), ("x", 15),
            ]
            emit = {"l": w_load, "c": w_chain, "x": x_quant}
            x_done = 3

            def drain(n):
                nonlocal x_done
                for _ in range(n):
                    if not pending:
                        return
                    kind, arg = pending.pop(0)
                    emit[kind](arg)
                    if kind == "x":
                        x_done = arg

            for mq in range(NMQ):
                for uh in range(2):
                    for u in range(uh * 8, uh * 8 + 8):
                        while u not in tt_tiles:
                            drain(1)
                        mm_group(mq, u)
                        drain(1)
                    # mquad mq+1's four x blocks must be emitted before
                    # its first group
                    if uh == 1 and mq + 1 < NMQ:
                        while x_done < (mq + 1) * 4 + 3:
                            drain(1)

    nc.compile()
    return nc


def _get_nc():
    if "nc" not in _cache:
        _cache["nc"] = _build()
    return _cache["nc"]


def run(x, weight, trace=False):
    """Run on 8 NeuronCores; returns (full output [B,S,D_OUT], results)."""
    from concourse.bass_utils import run_bass_kernel_spmd

    x = np.ascontiguousarray(np.asarray(x, dtype=np.float32))
    w = np.ascontiguousarray(np.asarray(weight, dtype=np.float32))
    assert x.shape == (B, S, D_IN) and w.shape == (D_OUT, D_IN)
    xf = x.reshape(M_TOT, D_IN)
    nc = _get_nc()
    in_maps = []
    for c in range(N_CORES):
        tq, wq = c % TQ, c // TQ
        in_maps.append({
            "x": np.ascontiguousarray(xf[tq * M_C:(tq + 1) * M_C]),
            "w": np.ascontiguousarray(w[wq * O_C:(wq + 1) * O_C]),
        })
    res = run_bass_kernel_spmd(nc, in_maps, core_ids=list(range(N_CORES)),
                               trace=trace)
    outf = np.empty((M_TOT, D_OUT), dtype=np.float32)
    for c in range(N_CORES):
        tq, wq = c % TQ, c // TQ
        outf[tq * M_C:(tq + 1) * M_C,
             wq * O_C:(wq + 1) * O_C] = res.results[c]["outT"].T
    return outf.reshape(B, S, D_OUT), res


def kernel(x, weight):
    out, _ = run(x, weight)
    return out
